# revision 17
# baseline (speedup 1.0000x reference)
"""Bass/Trainium2 kernel for nn_BerpXposMultiHeadedAttention (8-core SPMD).

Sharding: data-parallel over batch (4 batches x 2 cores) x tensor-parallel over
heads (4 heads per core).  Each core computes its 4 heads of flash-style xpos
attention for its batch plus the row-sharded partial out-projection; the host
sums the two partials per batch (the "all-reduce") and adds the output bias.

v2 design notes (sim-trace driven):
- DMA instruction count cut ~5x with batched 3D-AP transfers (one DMA per
  x-strip / table / packed-weight tensor); all issues ride the Pool sequencer
  (25ns/issue vs 565ns on SP, which serialized the first ~50us before).
- Weight matrices live in ONE packed DRAM tensor [128, 6144] so a single DMA
  loads every projection + the out-projection; constants load outside the
  repeat loop.
- The softmax denominator broadcast is a rank-1 PE matmul into the just-freed
  P@V PSUM tile (ones[1,64] x denom-row[1,512]) - no DRAM round trip.
- Causal trimming now also applies to exp and P@V streams (not just QK^T), so
  the above-diagonal pt memsets are gone entirely.
- out_proj chunks interleave one strip behind the flash loop instead of
  running as a serial tail.
- Engine split: PE matmuls; Act exp + PSUM evictions (exp_and_others serves
  both, no table reloads); DVE xpos combine + norm ops; GpSimd(Pool)
  SBUF-only memsets/muls + every dma_start.
"""

import sys

sys.path.insert(0, "/opt/trn_rl_repo")

import contextlib

import numpy as np

import concourse.bacc as bacc
import concourse.bass as bass
import concourse.tile as tile
from concourse import mybir
from concourse.bass_utils import run_bass_kernel_spmd

# Problem constants (hardcoded per the task contract).
B = 4
L = 2048
EMBED = 512
HEADS = 8
HD = 64
SCALE_BASE = 512
NEG = -1e9
N_CORES = 8
HPC = 4           # heads per core
TB = 512          # t-block (strip) width
NS = L // 128     # 16 s-chunks
NSTRIP = L // TB  # 4 strips
VW = 328          # v_aug tile width (4 heads x 65 + 68 pad)
WCOLS = 4096      # packed weight tensor width

F32 = mybir.dt.float32
F32R = mybir.dt.float32r
F16 = mybir.dt.float16
BF16 = mybir.dt.bfloat16

# Deinterleave permutation of a 64-wide head dim: evens then odds.
_PERM64 = np.concatenate([np.arange(0, HD, 2), np.arange(1, HD, 2)])


def _xpos_tables():
    """Host-side xpos cos/sin tables in the permuted [d, t] layout.

    Returns (csq, csk), each [128, 2L] float32: strip-interleaved packing
    [cos strip0 | sin strip0 | cos strip1 | ...] so one [128,1024] DVE mul
    covers both halves of a projection PSUM tile.  The 1/sqrt(HD) score
    scale is folded into the q pair.
    """
    d = HD
    base = ((np.arange(0, d, 2, dtype=np.float32) + np.float32(0.4 * d))
            / np.float32(1.4 * d)).astype(np.float32)                    # [32]
    min_pos = -(L // 2)
    power = (np.arange(min_pos, L + min_pos, dtype=np.float32)
             / np.float32(SCALE_BASE))                                   # [L]
    scale = (base[None, :] ** power[:, None]).astype(np.float32)         # [L, 32]
    half = d // 2
    inv_freq = (1.0 / (10000.0 ** (np.arange(half, dtype=np.float32) / half))
                ).astype(np.float32)
    sinusoid = np.arange(L, dtype=np.float32)[:, None] * inv_freq[None, :]
    sin = np.sin(sinusoid).astype(np.float32)
    cos = np.cos(sinusoid).astype(np.float32)

    def pack(cs, ss, fold):
        cs = (cs * fold).astype(np.float32)
        ss = (ss * fold).astype(np.float32)
        # permuted layout: rows 0:32 <- even orig dims, rows 32:64 <- odd.
        cos_p = np.concatenate([cs.T, cs.T], axis=0)      # [64, L]
        sin_p = np.concatenate([-ss.T, ss.T], axis=0)     # [64, L]
        cos2 = np.concatenate([cos_p, cos_p], axis=0)     # [128, L]
        sin2 = np.concatenate([sin_p, sin_p], axis=0)     # [128, L]
        # strip-interleave: [cos s0 | sin s0 | cos s1 | sin s1 | ...]
        out = np.empty((128, 2 * L), np.float32)
        for t in range(NSTRIP):
            out[:, 2 * t * TB:(2 * t + 1) * TB] = cos2[:, t * TB:(t + 1) * TB]
            out[:, (2 * t + 1) * TB:(2 * t + 2) * TB] = sin2[:, t * TB:(t + 1) * TB]
        return out

    inv_scale = (1.0 / scale).astype(np.float32)
    csq = pack(cos * scale, sin * scale, np.float32(HD ** -0.5))
    csk = pack(cos * inv_scale, sin * inv_scale, np.float32(1.0))
    return csq, csk


def _build_program(causal: bool, use_mask: bool, has_bias: bool, reps: int = 1):
    nc = bacc.Bacc("TRN2", target_bir_lowering=False, debug=False,
                   num_devices=N_CORES)

    # ---- DRAM I/O -------------------------------------------------------
    xqT = nc.dram_tensor("xqT", [513, L], F16, kind="ExternalInput")
    xkT = nc.dram_tensor("xkT", [513, L], F16, kind="ExternalInput")
    xvT = nc.dram_tensor("xvT", [513, L], F16, kind="ExternalInput")
    wAll = nc.dram_tensor("wAll", [128, WCOLS], BF16, kind="ExternalInput")
    wB = None
    if has_bias:
        wB = nc.dram_tensor("wB", [1, 768], BF16, kind="ExternalInput")
    csqD = nc.dram_tensor("csq", [128, 2 * L], F32, kind="ExternalInput")
    cskD = nc.dram_tensor("csk", [128, 2 * L], F32, kind="ExternalInput")
    triD = nc.dram_tensor("tri", [128, 128], F32, kind="ExternalInput")
    maskD = None
    if use_mask:
        maskD = nc.dram_tensor("maskT", [L, L], F32, kind="ExternalInput")
    outp = nc.dram_tensor("outp", [L, EMBED], F32, kind="ExternalOutput")

    permD = nc.dram_tensor("perm", [128, 128], F16, kind="ExternalInput")
    xin = {"q": xqT, "k": xkT, "v": xvT}
    # packed weight column offsets
    WOFF = {"qc": 0, "kc": 1024, "v": 2048, "o": 3072}
    BOFF = {"q": 0, "k": 256, "v": 512}

    with tile.TileContext(nc) as tc:
        with contextlib.ExitStack() as ctx:
            consts = ctx.enter_context(tc.tile_pool(name="consts", bufs=1))
            xpool = ctx.enter_context(tc.tile_pool(name="xpool", bufs=6))
            qkpool = ctx.enter_context(tc.tile_pool(name="qkpool", bufs=1))
            vpool = ctx.enter_context(tc.tile_pool(name="vpool", bufs=NS))
            tmp = ctx.enter_context(tc.tile_pool(name="tmp", bufs=2))
            tmp16 = ctx.enter_context(tc.tile_pool(name="tmp16", bufs=3))
            ptpool = ctx.enter_context(tc.tile_pool(name="ptpool", bufs=18))
            npool = ctx.enter_context(tc.tile_pool(name="npool", bufs=4))
            opool = ctx.enter_context(tc.tile_pool(name="opool", bufs=2))
            mpool = None
            if use_mask:
                mpool = ctx.enter_context(tc.tile_pool(name="mpool", bufs=NS + 2))
            ps_s = ctx.enter_context(tc.tile_pool(name="ps_s", bufs=3, space="PSUM"))
            ps_pv = ctx.enter_context(tc.tile_pool(name="ps_pv", bufs=2, space="PSUM"))

            # ---- preamble: constants loaded once, outside the rep loop ----
            # Split + ordered so the first-needed bytes land first: q weights
            # and strip-0 tables come ahead of everything else.
            wsb1 = consts.tile([128, 1024], BF16, tag="wsb1")   # qc
            nc.gpsimd.dma_start(wsb1[:], wAll[:, 0:1024])
            permT = consts.tile([128, 128], F16, tag="permT")
            nc.gpsimd.dma_start(permT[:], permD[:])
            tabs_q = []
            tabs_k = []
            for t in range(NSTRIP):
                tq = consts.tile([128, 1024], F32, tag=f"csq{t}")
                tk = consts.tile([128, 1024], F32, tag=f"csk{t}")
                tabs_q.append(tq)
                tabs_k.append(tk)
            nc.gpsimd.dma_start(tabs_q[0][:], csqD[:, 0:1024])
            nc.gpsimd.dma_start(tabs_k[0][:], cskD[:, 0:1024])
            wsb2 = consts.tile([128, 3072], BF16, tag="wsb2")   # kc|v|wo
            nc.gpsimd.dma_start(wsb2[:], wAll[:, 1024:WCOLS])
            for t in range(1, NSTRIP):
                nc.gpsimd.dma_start(tabs_q[t][:], csqD[:, t * 1024:(t + 1) * 1024])
                nc.gpsimd.dma_start(tabs_k[t][:], cskD[:, t * 1024:(t + 1) * 1024])
            tri_sb = consts.tile([128, 128], F32, tag="tri")
            if causal:
                nc.gpsimd.dma_start(tri_sb[:], triD[:])
            # ones row used as the rank-1 broadcast stationary; lives at
            # partition 64 to base-partition-match the denominator row.
            ones_f = consts.tile([65, 64], F32, tag="ones_f")
            nc.gpsimd.memset(ones_f[:], 1.0)
            ones1 = consts.tile([65, 64], F32R, tag="ones1")
            nc.vector.tensor_copy(ones1[:], ones_f[:])
            wb_sb = None
            ones_row = None
            if has_bias:
                wb_sb = consts.tile([1, 768], BF16, tag="wb")
                nc.gpsimd.dma_start(wb_sb[:], wB[:])
                ones_row = consts.tile([1, L], F16, tag="ones_row")
                nc.gpsimd.dma_start(ones_row[:], xqT[512:513, :])

            def wslice(nm, c, e=None):
                base = WOFF[nm] + c * 256
                w, boff = (wsb1, 0) if base < 1024 else (wsb2, 1024)
                base -= boff
                if e is None:
                    return w[:, base:base + 256]
                return w[:, base + e * 128:base + (e + 1) * 128]

            def body():
                attnT = [consts.tile([128, L], BF16, tag=f"attnT{c}",
                                     name=f"attnT{c}") for c in range(2)]
                qTt = [[None] * NSTRIP for _ in range(2)]  # [e][tb]
                kTt = [[None] * NSTRIP for _ in range(2)]
                vaug = [None] * NS
                xtiles = {}

                def load_x(nm, tb):
                    t = xpool.tile([128, 4 * TB], F16, tag="x",
                                   name=f"x{nm}{tb}")
                    base = xin[nm][:]
                    src = bass.AP(tensor=base.tensor,
                                  offset=base.offset + tb * TB,
                                  ap=[[L, 128], [128 * L, 4], [1, TB]])
                    dst = t[:].rearrange("p (c t) -> p c t", c=4)
                    nc.sync.dma_start(dst, src)
                    xtiles[(nm, tb)] = t

                def proj_qk(nm, tb, cstab, dst):
                    # One projection; the rotate-half partner is a partition
                    # permutation done as a rank-128 PE matmul (perm @ proj),
                    # replacing the second 4-matmul projection.
                    xs = xtiles.pop((nm, tb))
                    pss = []
                    for e in range(2):
                        ps = ps_s.tile([128, 1024], F32, tag="s",
                                       name=f"ps_{nm}{e}_{tb}")
                        for c in range(4):
                            nc.tensor.matmul(ps[:, 0:TB], wslice(nm + "c", c, e),
                                             xs[:, c * TB:(c + 1) * TB],
                                             start=(c == 0),
                                             stop=(c == 3 and not has_bias))
                        if has_bias:
                            bb = wb_sb[:, BOFF[nm] + e * 128:
                                       BOFF[nm] + (e + 1) * 128]
                            nc.tensor.matmul(ps[:, 0:TB], bb,
                                             ones_row[:, tb * TB:(tb + 1) * TB],
                                             start=False, stop=True)
                        pc = tmp16.tile([128, TB], F16, tag="pc",
                                        name=f"pc{nm}{e}{tb}")
                        nc.vector.tensor_copy(pc[:], ps[:, 0:TB])
                        pss.append((ps, pc))
                    for e in range(2):
                        ps, pc = pss[e]
                        nc.tensor.matmul(ps[:, TB:1024], permT[:], pc[:],
                                         start=True, stop=True)
                        t12 = tmp.tile([128, 1024], F32, tag="t12",
                                       name=f"t12{nm}{e}{tb}")
                        nc.vector.tensor_mul(t12[:], ps[:], cstab[tb][:])
                        ot = qkpool.tile([128, TB], F32R, tag=f"{nm}T{e}_{tb}",
                                         name=f"{nm}T{e}_{tb}")
                        nc.vector.tensor_add(ot[:], t12[:, 0:TB],
                                             t12[:, TB:1024])
                        dst[e][tb] = ot

                def proj_v(tb):
                    xs = xtiles.pop(("v", tb))
                    for j in range(4):
                        si = tb * 4 + j
                        ps = ps_pv.tile([128, TB], F32, tag="pv",
                                        name=f"ps_v{si}")
                        for c in range(4):
                            nc.tensor.matmul(
                                ps[:, 0:256],
                                xs[:, c * TB + j * 128:c * TB + (j + 1) * 128],
                                wslice("v", c),
                                start=(c == 0),
                                stop=(c == 3 and not has_bias))
                        if has_bias:
                            nc.tensor.matmul(
                                ps[:, 0:256],
                                ones_row[:, si * 128:(si + 1) * 128],
                                wb_sb[:, BOFF["v"]:BOFF["v"] + 256],
                                start=False, stop=True)
                        va = vpool.tile([128, VW], BF16, tag="vaug",
                                        name=f"vaug{si}")
                        va3 = va[:, 0:HPC * 65].rearrange("p (h c) -> p h c", c=65)
                        nc.vector.tensor_copy(
                            va3[:, :, 0:64],
                            ps[:, 0:256].rearrange("p (h d) -> p h d", d=64))
                        nc.gpsimd.memset(va3[:, :, 64:65], 1.0)
                        nc.gpsimd.memset(va[:, HPC * 65:VW], 0.0)
                        vaug[si] = va

                pending_norm = []

                def drain_one_norm():
                    if not pending_norm:
                        return
                    po, poc, ht, hr, T = pending_norm.pop(0)
                    # rank-1 broadcast of the denominator row into the freed
                    # P@V psum rows 0:64, then reciprocal + normalize.
                    nc.tensor.matmul(po[0:64, :], ones1[64:65, :],
                                     poc[64:65, :], start=True, stop=True)
                    rec = npool.tile([64, TB], F32, tag="rec",
                                     name=f"rec{T}x{ht}{hr}")
                    nc.vector.reciprocal(rec[:], po[0:64, :])
                    tcols = slice(T * TB, (T + 1) * TB)
                    if hr == 0:
                        nc.gpsimd.tensor_mul(attnT[ht][0:64, tcols],
                                             poc[0:64, :], rec[:])
                    else:
                        stag = npool.tile([64, TB], BF16, tag="stag",
                                          name=f"stag{T}x{ht}{hr}", bufs=2)
                        nc.vector.tensor_mul(stag[:], poc[0:64, :], rec[:])
                        nc.gpsimd.dma_start(attnT[ht][64:128, tcols], stag[:])

                def out_chunk(T):
                    osb = opool.tile([128, 4 * TB], F32, tag="osb",
                                     name=f"osb{T}")
                    for j in range(4):
                        tau = 4 * T + j
                        psB = ps_pv.tile([128, TB], F32, tag="pv",
                                         name=f"ps_o{tau}")
                        for c in range(2):
                            nc.tensor.matmul(
                                psB[:], attnT[c][:, tau * 128:(tau + 1) * 128],
                                wsb2[:, WOFF["o"] - 1024 + c * TB:
                                     WOFF["o"] - 1024 + (c + 1) * TB],
                                start=(c == 0), stop=(c == 1))
                        nc.scalar.copy(osb[:, j * TB:(j + 1) * TB], psB[:])
                    obase = outp[:]
                    dst = bass.AP(tensor=obase.tensor,
                                  offset=obase.offset + T * TB * EMBED,
                                  ap=[[EMBED, 128], [128 * EMBED, 4], [1, TB]])
                    nc.gpsimd.dma_start(
                        dst, osb[:].rearrange("p (j c) -> p j c", j=4))

                def s_phase(T, h, nsig, mtiles):
                    """Emit QK^T + exp for head h of strip T."""
                    ht, hr = h // 2, (h % 2) * 64
                    pts = []
                    offs = []
                    for g in range(nsig // 2):
                        ps2 = ps_s.tile([128, 1024], F32, tag="s",
                                        name=f"S{T}h{h}g{g}")
                        pt = ptpool.tile([128, 1024], BF16, tag="pt",
                                         name=f"P{T}h{h}g{g}")
                        diag_pair = causal and (2 * g + 1 - 4 * T) >= 0
                        for u in range(2):
                            sig = g * 2 + u
                            j = sig - 4 * T
                            coff = 0
                            off = 0
                            if causal and j >= 0:
                                # fp32r needs >=256-wide streams for full
                                # rate; off is the causally-valid start.
                                off = j * 128
                                coff = min(off, TB - 256)
                            nc.tensor.matmul(
                                ps2[:, u * TB + coff:(u + 1) * TB],
                                kTt[ht][sig // 4][hr:hr + 64,
                                                  (sig % 4) * 128:
                                                  (sig % 4 + 1) * 128],
                                qTt[ht][T][hr:hr + 64, coff:TB],
                                start=True, stop=True)
                            if causal and j >= 0:
                                sl = slice(u * TB + off, u * TB + off + 128)
                                nc.vector.tensor_add(ps2[:, sl], ps2[:, sl],
                                                     tri_sb[:])
                            if use_mask:
                                sl = slice(u * TB, (u + 1) * TB)
                                nc.vector.tensor_add(ps2[:, sl], ps2[:, sl],
                                                     mtiles[sig][:])
                            if diag_pair:
                                nc.scalar.activation(
                                    pt[:, u * TB + off:(u + 1) * TB],
                                    ps2[:, u * TB + off:(u + 1) * TB],
                                    mybir.ActivationFunctionType.Exp)
                            offs.append(off)
                        if not diag_pair:
                            nc.scalar.activation(
                                pt[:], ps2[:],
                                mybir.ActivationFunctionType.Exp)
                        pts.append(pt)
                    return pts, offs

                def pv_phase(T, h, nsig, pts, offs):
                    ht, hr = h // 2, (h % 2) * 64
                    # P@V: psum rows 0:64 attnU.T, row 64 sumexp
                    po = ps_pv.tile([128, TB], F32, tag="pv",
                                    name=f"po{T}h{h}")
                    for sig in range(nsig):
                        off = offs[sig]
                        nc.tensor.matmul(
                            po[:, off:TB],
                            vaug[sig][:, h * 65:h * 65 + 128],
                            pts[sig // 2][:, (sig % 2) * TB + off:
                                          (sig % 2 + 1) * TB],
                            start=(sig == 0), stop=(sig == nsig - 1))
                    poc = npool.tile([65, TB], F32R, tag="poc",
                                     name=f"poc{T}h{h}")
                    nc.vector.tensor_copy(poc[:], po[0:65, :])
                    pending_norm.append((po, poc, ht, hr, T))

                def flash_strip(T, fillers=()):
                    nsig = 4 * T + 4 if causal else NS
                    mtiles = None
                    if use_mask:
                        mtiles = []
                        for si in range(nsig):
                            mt = mpool.tile([128, TB], F32, tag="mask",
                                            name=f"m{T}_{si}")
                            nc.gpsimd.dma_start(
                                mt[:], maskD[si * 128:(si + 1) * 128,
                                             T * TB:(T + 1) * TB])
                            mtiles.append(mt)
                    # Software-pipelined: S(h+1) is emitted ahead of PV(h) so
                    # the PV matmuls never wait on exp of their own head, and
                    # next-strip projection pieces fill PE between heads.
                    cur = s_phase(T, 0, nsig, mtiles)
                    for h in range(HPC):
                        nxt = s_phase(T, h + 1, nsig, mtiles) \
                            if h + 1 < HPC else None
                        pv_phase(T, h, nsig, *cur)
                        if h >= 1:
                            drain_one_norm()
                        if h < len(fillers):
                            fillers[h]()
                        cur = nxt
                    drain_one_norm()
                    drain_one_norm()

                def mk_proj(nm, tb):
                    if nm == "q":
                        return lambda: proj_qk("q", tb, tabs_q, qTt)
                    if nm == "k":
                        return lambda: proj_qk("k", tb, tabs_k, kTt)

                    def fv():
                        proj_v(tb)
                        if tb + 1 < NSTRIP:
                            load_x("q", tb + 1)
                            load_x("v", tb + 1)
                            load_x("k", tb + 1)
                    return fv

                load_x("q", 0)
                load_x("v", 0)
                load_x("k", 0)
                proj_qk("q", 0, tabs_q, qTt)
                proj_v(0)
                load_x("q", 1)
                load_x("v", 1)
                load_x("k", 1)
                proj_qk("k", 0, tabs_k, kTt)
                proj_qk("q", 1, tabs_q, qTt)
                flash_strip(0, (mk_proj("v", 1), mk_proj("k", 1)))
                for tb in range(1, NSTRIP):
                    fillers = [lambda t=tb: out_chunk(t - 1)]
                    if tb + 1 < NSTRIP:
                        fillers += [mk_proj("q", tb + 1),
                                    mk_proj("v", tb + 1),
                                    mk_proj("k", tb + 1)]
                    flash_strip(tb, tuple(fillers))
                out_chunk(NSTRIP - 1)

            if reps > 1:
                with tc.For_i(0, reps, 1,
                              hint_engines=(mybir.EngineType.PE,
                                            mybir.EngineType.Activation,
                                            mybir.EngineType.DVE,
                                            mybir.EngineType.SP,
                                            mybir.EngineType.Pool)):
                    body()
            else:
                body()

    nc.compile()
    return nc


_PROGRAM_CACHE = {}


def get_program(causal: bool, use_mask: bool, has_bias: bool, reps: int = 1):
    key = (causal, use_mask, has_bias, reps)
    if key not in _PROGRAM_CACHE:
        _PROGRAM_CACHE[key] = _build_program(causal, use_mask, has_bias, reps)
    return _PROGRAM_CACHE[key]


def _prep_in_maps(query, key, value, key_padding_mask, attn_mask,
                  Wq, bq, Wk, bk, Wv, bv, Wo, bo, use_mask, has_bias):
    """Build the 8 per-core input dicts."""
    import ml_dtypes
    csq, csk = _xpos_tables()
    tri = np.where(np.arange(128)[None, :] >= np.arange(128)[:, None],
                   np.float32(0.0), np.float32(NEG)).astype(np.float32)

    def aug_x(x):
        a = np.empty((513, L), np.float16)
        a[0:512] = np.asarray(x, np.float32).T.astype(np.float16)
        a[512] = np.float16(1.0)
        return a

    xqTs = [aug_x(query[b]) for b in range(B)]
    xkTs = [aug_x(key[b]) for b in range(B)]
    xvTs = [aug_x(value[b]) for b in range(B)]

    masks = None
    if use_mask:
        am = np.asarray(attn_mask, np.float32)
        kp = np.asarray(key_padding_mask)
        masks = []
        for b in range(B):
            m = am.copy()
            if kp[b].any():
                m = m + np.where(kp[b], np.float32(-1e30),
                                 np.float32(0.0))[None, :]
            masks.append(np.ascontiguousarray(m.T.astype(np.float32)))

    Wq = np.asarray(Wq, np.float32); bq = np.asarray(bq, np.float32)
    Wk = np.asarray(Wk, np.float32); bk = np.asarray(bk, np.float32)
    Wv = np.asarray(Wv, np.float32); bv = np.asarray(bv, np.float32)
    Wo = np.asarray(Wo, np.float32)

    in_maps = []
    for core in range(N_CORES):
        b, hg = core // 2, core % 2
        hs = hg * HPC
        idx_p = np.concatenate(
            [hs * HD + hl * HD + _PERM64 for hl in range(HPC)])
        # sin-projection rows: within each head's 64-block, row r <- r XOR 32
        xor = (np.arange(256).reshape(HPC, HD)[:, (np.arange(HD) ^ 32)]
               ).reshape(-1)
        idx_s = idx_p[xor]
        idx_v = hs * HD + np.arange(HPC * HD)

        # packed weights: wAll[k, i*1024 + c*256 + j] = W[idx[j], c*128 + k]
        wall = np.empty((128, WCOLS), np.float32)
        for i, (W, idx) in enumerate([(Wq, idx_p), (Wk, idx_p),
                                      (Wv, idx_v)]):
            blk = W[idx, :]                    # [256 out, 512 in]
            for c in range(4):
                wall[:, i * 1024 + c * 256:(i * 1024 + (c + 1) * 256)] = \
                    blk[:, c * 128:(c + 1) * 128].T
        woT = Wo[:, idx_v].T                   # [256 v, 512 embed]
        for c2 in range(2):
            wall[:, 3072 + c2 * 512:3072 + (c2 + 1) * 512] = \
                woT[c2 * 128:(c2 + 1) * 128, :]

        perm = np.zeros((128, 128), np.float16)
        for mm in range(128):
            perm[(mm // 64) * 64 + ((mm % 64) ^ 32), mm] = np.float16(1.0)
        m = {
            "xqT": xqTs[b], "xkT": xkTs[b], "xvT": xvTs[b],
            "wAll": wall.astype(ml_dtypes.bfloat16),
            "csq": csq, "csk": csk,
            "tri": tri, "perm": perm,
        }
        if has_bias:
            wbias = np.empty((1, 768), np.float32)
            wbias[0, 0:256] = bq[idx_p]
            wbias[0, 256:512] = bk[idx_p]
            wbias[0, 512:768] = bv[idx_v]
            m["wB"] = wbias.astype(ml_dtypes.bfloat16)
        if use_mask:
            m["maskT"] = masks[b]
        in_maps.append(m)
    return in_maps


def classify_mask(attn_mask, key_padding_mask):
    am = np.asarray(attn_mask, np.float32)
    kp = np.asarray(key_padding_mask)
    if not kp.any():
        causal = np.where(
            np.tril(np.ones((L, L), bool)), np.float32(0.0),
            np.float32(NEG)).astype(np.float32)
        if np.array_equal(am, causal):
            return True, False
        if not am.any():
            return False, False
    return False, True


def kernel(query, key, value, key_padding_mask, attn_mask,
           Wq, bq, Wk, bk, Wv, bv, Wo, bo):
    causal, use_mask = classify_mask(attn_mask, key_padding_mask)
    has_bias = bool(np.asarray(bq).any() or np.asarray(bk).any()
                    or np.asarray(bv).any())
    nc = get_program(causal, use_mask, has_bias, reps=1)
    in_maps = _prep_in_maps(query, key, value, key_padding_mask, attn_mask,
                            Wq, bq, Wk, bk, Wv, bv, Wo, bo, use_mask, has_bias)
    res = run_bass_kernel_spmd(nc, in_maps, list(range(N_CORES)))
    bo = np.asarray(bo, np.float32)
    out = np.empty((B, L, EMBED), np.float32)
    for b in range(B):
        out[b] = (res.results[2 * b]["outp"]
                  + res.results[2 * b + 1]["outp"] + bo[None, :])
    return out


# revision 27
# speedup vs baseline: 1.6824x; 1.6824x over previous
"""Bass/Trainium2 kernel for nn_BerpXposMultiHeadedAttention (8-core SPMD).

Sharding: data-parallel over batch (4 batches x 2 cores) x tensor-parallel over
heads (4 heads per core).  Each core computes its 4 heads of flash-style xpos
attention for its batch plus the row-sharded partial out-projection; the host
sums the two partials per batch (the "all-reduce") and adds the output bias.

Design notes (cost-model-trace driven; sim span 125us/rep vs 164us baseline):
- ~35 DMA instructions per rep (baseline ~152) via batched 3D-AP transfers;
  per-rep transfers (x strips) ride the SP queue while constants/stores ride
  the Pool queue, so strip-0 inputs never wait behind preamble bytes.
  Constants (weights/tables/masks) load once, outside the repeat loop.
- xpos needs q*cos + rot(q)*sin.  rot() in the deinterleaved row layout is a
  partition permutation, done as ONE rank-128 PE matmul against a constant
  permutation matrix (fp16) instead of a second 4-matmul projection: PE
  stream work drops ~12% and weight DMA bytes drop 2/3.
- The softmax denominator broadcast is a rank-1 PE matmul into the just-freed
  P@V PSUM tile (ones[1,64] x denom-row[1,512]) - no DRAM round trip (the
  baseline bounced every denominator through DRAM).
- Causal trimming applies to QK^T, exp, and P@V streams alike; the
  within-block triangle is zeroed multiplicatively on exp(scores) with a 0/1
  bf16 mask on the otherwise-idle GpSimd engine (keeps the hot S-chain
  matmul->exp with no DVE hop in between).
- Flash is software-pipelined: S(h+1) is emitted ahead of P@V(h) so P@V never
  waits on its own head's exp; next-strip projection pieces act as PE filler
  between head phases; out_proj chunks run one strip behind instead of as a
  serial tail (the last chunk ships per-half with Act evictions).
- Engine split: PE matmuls; Act exp + projection fp16 evictions; DVE xpos
  combine + norm ops + mid out-proj evictions; GpSimd triangle masks,
  memsets, and most dma_start issues (25ns/issue vs 565ns on SP).
"""

import sys

sys.path.insert(0, "/opt/trn_rl_repo")

import contextlib

import numpy as np

import concourse.bacc as bacc
import concourse.bass as bass
import concourse.tile as tile
from concourse import mybir
from concourse.bass_utils import run_bass_kernel_spmd

# Problem constants (hardcoded per the task contract).
B = 4
L = 2048
EMBED = 512
HEADS = 8
HD = 64
SCALE_BASE = 512
NEG = -1e9
N_CORES = 8
HPC = 4           # heads per core
TB = 512          # t-block (strip) width
NS = L // 128     # 16 s-chunks
NSTRIP = L // TB  # 4 strips
VW = 328          # v_aug tile width (4 heads x 65 + 68 pad)
WCOLS = 4096      # packed weight tensor width

F32 = mybir.dt.float32
F32R = mybir.dt.float32r
F16 = mybir.dt.float16
BF16 = mybir.dt.bfloat16

# Deinterleave permutation of a 64-wide head dim: evens then odds.
_PERM64 = np.concatenate([np.arange(0, HD, 2), np.arange(1, HD, 2)])


def _xpos_tables():
    """Host-side xpos cos/sin tables in the permuted [d, t] layout.

    Returns (csq, csk), each [128, 2L] float32: strip-interleaved packing
    [cos strip0 | sin strip0 | cos strip1 | ...] so one [128,1024] DVE mul
    covers both halves of a projection PSUM tile.  The 1/sqrt(HD) score
    scale is folded into the q pair.
    """
    d = HD
    base = ((np.arange(0, d, 2, dtype=np.float32) + np.float32(0.4 * d))
            / np.float32(1.4 * d)).astype(np.float32)                    # [32]
    min_pos = -(L // 2)
    power = (np.arange(min_pos, L + min_pos, dtype=np.float32)
             / np.float32(SCALE_BASE))                                   # [L]
    scale = (base[None, :] ** power[:, None]).astype(np.float32)         # [L, 32]
    half = d // 2
    inv_freq = (1.0 / (10000.0 ** (np.arange(half, dtype=np.float32) / half))
                ).astype(np.float32)
    sinusoid = np.arange(L, dtype=np.float32)[:, None] * inv_freq[None, :]
    sin = np.sin(sinusoid).astype(np.float32)
    cos = np.cos(sinusoid).astype(np.float32)

    def pack(cs, ss, fold):
        cs = (cs * fold).astype(np.float32)
        ss = (ss * fold).astype(np.float32)
        # permuted layout: rows 0:32 <- even orig dims, rows 32:64 <- odd.
        cos_p = np.concatenate([cs.T, cs.T], axis=0)      # [64, L]
        sin_p = np.concatenate([-ss.T, ss.T], axis=0)     # [64, L]
        cos2 = np.concatenate([cos_p, cos_p], axis=0)     # [128, L]
        sin2 = np.concatenate([sin_p, sin_p], axis=0)     # [128, L]
        # strip-interleave: [cos s0 | sin s0 | cos s1 | sin s1 | ...]
        out = np.empty((128, 2 * L), np.float32)
        for t in range(NSTRIP):
            out[:, 2 * t * TB:(2 * t + 1) * TB] = cos2[:, t * TB:(t + 1) * TB]
            out[:, (2 * t + 1) * TB:(2 * t + 2) * TB] = sin2[:, t * TB:(t + 1) * TB]
        return out

    inv_scale = (1.0 / scale).astype(np.float32)
    csq = pack(cos * scale, sin * scale, np.float32(HD ** -0.5))
    csk = pack(cos * inv_scale, sin * inv_scale, np.float32(1.0))
    return csq, csk


def _build_program(causal: bool, use_mask: bool, has_bias: bool, reps: int = 1):
    nc = bacc.Bacc("TRN2", target_bir_lowering=False, debug=False,
                   num_devices=N_CORES)

    # ---- DRAM I/O -------------------------------------------------------
    xqT = nc.dram_tensor("xqT", [513, L], F16, kind="ExternalInput")
    xkT = nc.dram_tensor("xkT", [513, L], F16, kind="ExternalInput")
    xvT = nc.dram_tensor("xvT", [513, L], F16, kind="ExternalInput")
    wAll = nc.dram_tensor("wAll", [128, WCOLS], BF16, kind="ExternalInput")
    wB = None
    if has_bias:
        wB = nc.dram_tensor("wB", [1, 768], BF16, kind="ExternalInput")
    csqD = nc.dram_tensor("csq", [128, 2 * L], F32, kind="ExternalInput")
    cskD = nc.dram_tensor("csk", [128, 2 * L], F32, kind="ExternalInput")
    triD = nc.dram_tensor("tri", [128, 128], BF16, kind="ExternalInput")
    maskD = None
    if use_mask:
        maskD = nc.dram_tensor("maskT", [L, L], F32, kind="ExternalInput")
    outp = nc.dram_tensor("outp", [L, EMBED], F32, kind="ExternalOutput")

    permD = nc.dram_tensor("perm", [128, 128], F16, kind="ExternalInput")
    xin = {"q": xqT, "k": xkT, "v": xvT}
    # packed weight column offsets
    WOFF = {"qc": 0, "kc": 1024, "v": 2048, "o": 3072}
    BOFF = {"q": 0, "k": 256, "v": 512}

    with tile.TileContext(nc) as tc:
        with contextlib.ExitStack() as ctx:
            consts = ctx.enter_context(tc.tile_pool(name="consts", bufs=1))
            xpool = ctx.enter_context(tc.tile_pool(name="xpool", bufs=6))
            qkpool = ctx.enter_context(tc.tile_pool(name="qkpool", bufs=1))
            vpool = ctx.enter_context(tc.tile_pool(name="vpool", bufs=NS))
            tmp = ctx.enter_context(tc.tile_pool(name="tmp", bufs=2))
            tmp16 = ctx.enter_context(tc.tile_pool(name="tmp16", bufs=3))
            ptpool = ctx.enter_context(tc.tile_pool(name="ptpool",
                                                    bufs=10 if use_mask else 18))
            npool = ctx.enter_context(tc.tile_pool(name="npool", bufs=4))
            opool = ctx.enter_context(tc.tile_pool(name="opool", bufs=2))
            mpool = None
            if use_mask:
                mpool = ctx.enter_context(tc.tile_pool(name="mpool", bufs=NS + 2))
            ps_s = ctx.enter_context(tc.tile_pool(name="ps_s", bufs=3, space="PSUM"))
            ps_pv = ctx.enter_context(tc.tile_pool(name="ps_pv", bufs=2, space="PSUM"))

            # ---- preamble: constants loaded once, outside the rep loop ----
            # Split + ordered so the first-needed bytes land first: q weights
            # and strip-0 tables come ahead of everything else.
            wsb1 = consts.tile([128, 1024], BF16, tag="wsb1")   # qc
            nc.gpsimd.dma_start(wsb1[:], wAll[:, 0:1024])
            permT = consts.tile([128, 128], F16, tag="permT")
            nc.gpsimd.dma_start(permT[:], permD[:])
            tabs_q = []
            tabs_k = []
            for t in range(NSTRIP):
                tq = consts.tile([128, 1024], F32, tag=f"csq{t}")
                tk = consts.tile([128, 1024], F32, tag=f"csk{t}")
                tabs_q.append(tq)
                tabs_k.append(tk)
            nc.gpsimd.dma_start(tabs_q[0][:], csqD[:, 0:1024])
            wkc = consts.tile([128, 1024], BF16, tag="wkc")     # kc
            nc.gpsimd.dma_start(wkc[:], wAll[:, 1024:2048])
            nc.gpsimd.dma_start(tabs_k[0][:], cskD[:, 0:1024])
            wvo = consts.tile([128, 2048], BF16, tag="wvo")     # v|wo
            nc.gpsimd.dma_start(wvo[:], wAll[:, 2048:WCOLS])
            for t in range(1, NSTRIP):
                nc.gpsimd.dma_start(tabs_q[t][:], csqD[:, t * 1024:(t + 1) * 1024])
                nc.gpsimd.dma_start(tabs_k[t][:], cskD[:, t * 1024:(t + 1) * 1024])
            tri_sb = consts.tile([128, 128], BF16, tag="tri")
            if causal:
                nc.gpsimd.dma_start(tri_sb[:], triD[:])
            # ones row used as the rank-1 broadcast stationary; lives at
            # partition 64 to base-partition-match the denominator row.
            ones_f = consts.tile([65, 64], F32, tag="ones_f")
            nc.gpsimd.memset(ones_f[:], 1.0)
            ones1 = consts.tile([65, 64], F32R, tag="ones1")
            nc.vector.tensor_copy(ones1[:], ones_f[:])
            wb_sb = None
            ones_row = None
            if has_bias:
                wb_sb = consts.tile([1, 768], BF16, tag="wb")
                nc.gpsimd.dma_start(wb_sb[:], wB[:])
                ones_row = consts.tile([1, L], F16, tag="ones_row")
                nc.gpsimd.dma_start(ones_row[:], xqT[512:513, :])

            def wslice(nm, c, e=None):
                base = WOFF[nm] + c * 256
                w, boff = {"qc": (wsb1, 0), "kc": (wkc, 1024),
                           "v": (wvo, 2048)}[nm]
                base -= boff
                if e is None:
                    return w[:, base:base + 256]
                return w[:, base + e * 128:base + (e + 1) * 128]

            def body():
                attnT = [consts.tile([128, L], BF16, tag=f"attnT{c}",
                                     name=f"attnT{c}") for c in range(2)]
                qTt = [[None] * NSTRIP for _ in range(2)]  # [e][tb]
                kTt = [[None] * NSTRIP for _ in range(2)]
                vaug = [None] * NS
                xtiles = {}

                def load_x(nm, tb):
                    t = xpool.tile([128, 4 * TB], F16, tag="x",
                                   name=f"x{nm}{tb}")
                    base = xin[nm][:]
                    src = bass.AP(tensor=base.tensor,
                                  offset=base.offset + tb * TB,
                                  ap=[[L, 128], [128 * L, 4], [1, TB]])
                    dst = t[:].rearrange("p (c t) -> p c t", c=4)
                    nc.sync.dma_start(dst, src)
                    xtiles[(nm, tb)] = t

                def proj_qk(nm, tb, cstab, dst):
                    # One projection; the rotate-half partner is a partition
                    # permutation done as a rank-128 PE matmul (perm @ proj),
                    # replacing the second 4-matmul projection.
                    xs = xtiles.pop((nm, tb))
                    pss = []
                    for e in range(2):
                        ps = ps_s.tile([128, 1024], F32, tag="s",
                                       name=f"ps_{nm}{e}_{tb}")
                        for c in range(4):
                            nc.tensor.matmul(ps[:, 0:TB], wslice(nm + "c", c, e),
                                             xs[:, c * TB:(c + 1) * TB],
                                             start=(c == 0),
                                             stop=(c == 3 and not has_bias))
                        if has_bias:
                            bb = wb_sb[:, BOFF[nm] + e * 128:
                                       BOFF[nm] + (e + 1) * 128]
                            nc.tensor.matmul(ps[:, 0:TB], bb,
                                             ones_row[:, tb * TB:(tb + 1) * TB],
                                             start=False, stop=True)
                        pc = tmp16.tile([128, TB], F16, tag="pc",
                                        name=f"pc{nm}{e}{tb}")
                        nc.scalar.copy(pc[:], ps[:, 0:TB])
                        pss.append((ps, pc))
                    for e in range(2):
                        ps, pc = pss[e]
                        nc.tensor.matmul(ps[:, TB:1024], permT[:], pc[:],
                                         start=True, stop=True)
                        t12 = tmp.tile([128, 1024], F32, tag="t12",
                                       name=f"t12{nm}{e}{tb}")
                        nc.vector.tensor_mul(t12[:], ps[:], cstab[tb][:])
                        ot = qkpool.tile([128, TB], F32R, tag=f"{nm}T{e}_{tb}",
                                         name=f"{nm}T{e}_{tb}")
                        nc.vector.tensor_add(ot[:], t12[:, 0:TB],
                                             t12[:, TB:1024])
                        dst[e][tb] = ot

                def proj_v(tb):
                    xs = xtiles.pop(("v", tb))
                    for j in range(4):
                        si = tb * 4 + j
                        ps = ps_pv.tile([128, TB], F32, tag="pv",
                                        name=f"ps_v{si}")
                        for c in range(4):
                            nc.tensor.matmul(
                                ps[:, 0:256],
                                xs[:, c * TB + j * 128:c * TB + (j + 1) * 128],
                                wslice("v", c),
                                start=(c == 0),
                                stop=(c == 3 and not has_bias))
                        if has_bias:
                            nc.tensor.matmul(
                                ps[:, 0:256],
                                ones_row[:, si * 128:(si + 1) * 128],
                                wb_sb[:, BOFF["v"]:BOFF["v"] + 256],
                                start=False, stop=True)
                        va = vpool.tile([128, VW], BF16, tag="vaug",
                                        name=f"vaug{si}")
                        va3 = va[:, 0:HPC * 65].rearrange("p (h c) -> p h c", c=65)
                        nc.vector.tensor_copy(
                            va3[:, :, 0:64],
                            ps[:, 0:256].rearrange("p (h d) -> p h d", d=64))
                        nc.gpsimd.memset(va3[:, :, 64:65], 1.0)
                        nc.gpsimd.memset(va[:, HPC * 65:VW], 0.0)
                        vaug[si] = va

                pending_norm = []

                def drain_one_norm():
                    if not pending_norm:
                        return
                    po, poc, ht, hr, T = pending_norm.pop(0)
                    # rank-1 broadcast of the denominator row into the freed
                    # P@V psum rows 0:64, then reciprocal + normalize.
                    nc.tensor.matmul(po[0:64, :], ones1[64:65, :],
                                     poc[64:65, :], start=True, stop=True)
                    rec = npool.tile([64, TB], F32, tag="rec",
                                     name=f"rec{T}x{ht}{hr}")
                    nc.vector.reciprocal(rec[:], po[0:64, :])
                    tcols = slice(T * TB, (T + 1) * TB)
                    if hr == 0:
                        nc.gpsimd.tensor_mul(attnT[ht][0:64, tcols],
                                             poc[0:64, :], rec[:])
                    else:
                        stag = npool.tile([64, TB], BF16, tag="stag",
                                          name=f"stag{T}x{ht}{hr}", bufs=2)
                        nc.vector.tensor_mul(stag[:], poc[0:64, :], rec[:])
                        nc.gpsimd.dma_start(attnT[ht][64:128, tcols], stag[:])

                def out_chunk(T):
                    last = T == NSTRIP - 1
                    osb = opool.tile([128, 4 * TB], F32, tag="osb",
                                     name=f"osb{T}")
                    obase = outp[:]
                    for j in range(4):
                        tau = 4 * T + j
                        psB = ps_pv.tile([128, TB], F32, tag="pv",
                                         name=f"ps_o{tau}")
                        for c in range(2):
                            nc.tensor.matmul(
                                psB[:], attnT[c][:, tau * 128:(tau + 1) * 128],
                                wvo[:, WOFF["o"] - 2048 + c * TB:
                                    WOFF["o"] - 2048 + (c + 1) * TB],
                                start=(c == 0), stop=(c == 1))
                        if last:
                            # tail: Act is idle; ship each half as soon as
                            # its evictions land so DMA overlaps the copies.
                            nc.scalar.copy(osb[:, j * TB:(j + 1) * TB], psB[:])
                            if j % 2 == 1:
                                dst = bass.AP(
                                    tensor=obase.tensor,
                                    offset=obase.offset
                                    + (T * TB + (j - 1) * 128) * EMBED,
                                    ap=[[EMBED, 128], [128 * EMBED, 2],
                                        [1, TB]])
                                nc.gpsimd.dma_start(
                                    dst, osb[:, (j - 1) * TB:(j + 1) * TB]
                                    .rearrange("p (i c) -> p i c", i=2))
                        else:
                            nc.vector.tensor_copy(osb[:, j * TB:(j + 1) * TB],
                                                  psB[:])
                    if not last:
                        dst = bass.AP(
                            tensor=obase.tensor,
                            offset=obase.offset + T * TB * EMBED,
                            ap=[[EMBED, 128], [128 * EMBED, 4], [1, TB]])
                        nc.gpsimd.dma_start(
                            dst, osb[:].rearrange("p (j c) -> p j c", j=4))

                def s_phase(T, h, nsig, mtiles):
                    """Emit QK^T + exp for head h of strip T."""
                    ht, hr = h // 2, (h % 2) * 64
                    pts = []
                    offs = []
                    for g in range(nsig // 2):
                        ps2 = ps_s.tile([128, 1024], F32, tag="s",
                                        name=f"S{T}h{h}g{g}")
                        pt = ptpool.tile([128, 1024], BF16, tag="pt",
                                         name=f"P{T}h{h}g{g}")
                        diag_pair = causal and (2 * g + 1 - 4 * T) >= 0
                        for u in range(2):
                            sig = g * 2 + u
                            j = sig - 4 * T
                            coff = 0
                            off = 0
                            if causal and j >= 0:
                                # fp32r needs >=256-wide streams for full
                                # rate; off is the causally-valid start.
                                off = j * 128
                                coff = min(off, TB - 256)
                            nc.tensor.matmul(
                                ps2[:, u * TB + coff:(u + 1) * TB],
                                kTt[ht][sig // 4][hr:hr + 64,
                                                  (sig % 4) * 128:
                                                  (sig % 4 + 1) * 128],
                                qTt[ht][T][hr:hr + 64, coff:TB],
                                start=True, stop=True)
                            if use_mask:
                                sl = slice(u * TB, (u + 1) * TB)
                                nc.vector.tensor_add(ps2[:, sl], ps2[:, sl],
                                                     mtiles[sig][:])
                            if diag_pair:
                                nc.scalar.activation(
                                    pt[:, u * TB + off:(u + 1) * TB],
                                    ps2[:, u * TB + off:(u + 1) * TB],
                                    mybir.ActivationFunctionType.Exp)
                            if causal and j >= 0:
                                # zero the within-block upper triangle of
                                # exp(scores) via a 0/1 mask (idle Pool eng).
                                sl = slice(u * TB + off, u * TB + off + 128)
                                nc.gpsimd.tensor_mul(pt[:, sl], pt[:, sl],
                                                     tri_sb[:])
                            offs.append(off)
                        if not diag_pair:
                            nc.scalar.activation(
                                pt[:], ps2[:],
                                mybir.ActivationFunctionType.Exp)
                        pts.append(pt)
                    return pts, offs

                def pv_phase(T, h, nsig, pts, offs):
                    ht, hr = h // 2, (h % 2) * 64
                    # P@V: psum rows 0:64 attnU.T, row 64 sumexp
                    po = ps_pv.tile([128, TB], F32, tag="pv",
                                    name=f"po{T}h{h}")
                    for sig in range(nsig):
                        off = offs[sig]
                        nc.tensor.matmul(
                            po[:, off:TB],
                            vaug[sig][:, h * 65:h * 65 + 128],
                            pts[sig // 2][:, (sig % 2) * TB + off:
                                          (sig % 2 + 1) * TB],
                            start=(sig == 0), stop=(sig == nsig - 1))
                    poc = npool.tile([65, TB], F32R, tag="poc",
                                     name=f"poc{T}h{h}")
                    nc.vector.tensor_copy(poc[:], po[0:65, :])
                    pending_norm.append((po, poc, ht, hr, T))

                def flash_strip(T, fillers=()):
                    nsig = 4 * T + 4 if causal else NS
                    mtiles = None
                    if use_mask:
                        mtiles = []
                        for si in range(nsig):
                            mt = mpool.tile([128, TB], BF16, tag="mask",
                                            name=f"m{T}_{si}")
                            nc.gpsimd.dma_start(
                                mt[:], maskD[si * 128:(si + 1) * 128,
                                             T * TB:(T + 1) * TB])
                            mtiles.append(mt)
                    # Software-pipelined: S(h+1) is emitted ahead of PV(h) so
                    # the PV matmuls never wait on exp of their own head, and
                    # next-strip projection pieces fill PE between heads.
                    # (mask path: no lookahead, to fit mask tiles in SBUF)
                    la = 0 if use_mask else 1
                    cur = s_phase(T, 0, nsig, mtiles)
                    for h in range(HPC):
                        nxt = s_phase(T, h + la, nsig, mtiles) \
                            if 0 < h + la < HPC else None
                        if h >= 2:
                            drain_one_norm()
                        if la == 0 and h > 0:
                            cur = nxt
                        pv_phase(T, h, nsig, *cur)
                        if h < len(fillers):
                            fillers[h]()
                        if la:
                            cur = nxt
                    drain_one_norm()
                    drain_one_norm()

                def mk_proj(nm, tb):
                    if nm == "q":
                        return lambda: proj_qk("q", tb, tabs_q, qTt)
                    if nm == "k":
                        return lambda: proj_qk("k", tb, tabs_k, kTt)

                    def fv():
                        proj_v(tb)
                        if tb + 1 < NSTRIP:
                            load_x("q", tb + 1)
                            load_x("v", tb + 1)
                            load_x("k", tb + 1)
                    return fv

                if causal:
                    load_x("q", 0)
                    load_x("k", 0)
                    load_x("v", 0)
                    proj_qk("q", 0, tabs_q, qTt)
                    proj_qk("k", 0, tabs_k, kTt)
                    proj_v(0)
                    load_x("q", 1)
                    load_x("v", 1)
                    load_x("k", 1)
                    proj_qk("q", 1, tabs_q, qTt)
                    flash_strip(0, (mk_proj("v", 1), mk_proj("k", 1)))
                    for tb in range(1, NSTRIP):
                        fillers = [lambda t=tb: out_chunk(t - 1)]
                        if tb + 1 < NSTRIP:
                            fillers += [mk_proj("q", tb + 1),
                                        mk_proj("v", tb + 1),
                                        mk_proj("k", tb + 1)]
                        flash_strip(tb, tuple(fillers))
                    out_chunk(NSTRIP - 1)
                else:
                    # Non-causal: every strip's flash reads all k/v strips,
                    # so run all projections first (no fillers).
                    for tb in range(NSTRIP):
                        load_x("q", tb)
                        load_x("k", tb)
                        load_x("v", tb)
                        proj_qk("q", tb, tabs_q, qTt)
                        proj_qk("k", tb, tabs_k, kTt)
                        proj_v(tb)
                    for tb in range(NSTRIP):
                        if tb > 0:
                            out_chunk(tb - 1)
                        flash_strip(tb)
                    out_chunk(NSTRIP - 1)

            if reps > 1:
                with tc.For_i(0, reps, 1,
                              hint_engines=(mybir.EngineType.PE,
                                            mybir.EngineType.Activation,
                                            mybir.EngineType.DVE,
                                            mybir.EngineType.SP,
                                            mybir.EngineType.Pool)):
                    body()
            else:
                body()

    nc.compile()
    return nc


_PROGRAM_CACHE = {}


def get_program(causal: bool, use_mask: bool, has_bias: bool, reps: int = 1):
    key = (causal, use_mask, has_bias, reps)
    if key not in _PROGRAM_CACHE:
        _PROGRAM_CACHE[key] = _build_program(causal, use_mask, has_bias, reps)
    return _PROGRAM_CACHE[key]


def _prep_in_maps(query, key, value, key_padding_mask, attn_mask,
                  Wq, bq, Wk, bk, Wv, bv, Wo, bo, use_mask, has_bias):
    """Build the 8 per-core input dicts."""
    import ml_dtypes
    csq, csk = _xpos_tables()
    tri = (np.arange(128)[None, :] >= np.arange(128)[:, None])

    def aug_x(x):
        a = np.empty((513, L), np.float16)
        a[0:512] = np.asarray(x, np.float32).T.astype(np.float16)
        a[512] = np.float16(1.0)
        return a

    xqTs = [aug_x(query[b]) for b in range(B)]
    xkTs = [aug_x(key[b]) for b in range(B)]
    xvTs = [aug_x(value[b]) for b in range(B)]

    masks = None
    if use_mask:
        am = np.asarray(attn_mask, np.float32)
        kp = np.asarray(key_padding_mask)
        masks = []
        for b in range(B):
            m = am.copy()
            if kp[b].any():
                m = m + np.where(kp[b], np.float32(-1e30),
                                 np.float32(0.0))[None, :]
            masks.append(np.ascontiguousarray(m.T.astype(np.float32)))

    Wq = np.asarray(Wq, np.float32); bq = np.asarray(bq, np.float32)
    Wk = np.asarray(Wk, np.float32); bk = np.asarray(bk, np.float32)
    Wv = np.asarray(Wv, np.float32); bv = np.asarray(bv, np.float32)
    Wo = np.asarray(Wo, np.float32)

    in_maps = []
    for core in range(N_CORES):
        b, hg = core // 2, core % 2
        hs = hg * HPC
        idx_p = np.concatenate(
            [hs * HD + hl * HD + _PERM64 for hl in range(HPC)])
        # sin-projection rows: within each head's 64-block, row r <- r XOR 32
        xor = (np.arange(256).reshape(HPC, HD)[:, (np.arange(HD) ^ 32)]
               ).reshape(-1)
        idx_s = idx_p[xor]
        idx_v = hs * HD + np.arange(HPC * HD)

        # packed weights: wAll[k, i*1024 + c*256 + j] = W[idx[j], c*128 + k]
        wall = np.empty((128, WCOLS), np.float32)
        for i, (W, idx) in enumerate([(Wq, idx_p), (Wk, idx_p),
                                      (Wv, idx_v)]):
            blk = W[idx, :]                    # [256 out, 512 in]
            for c in range(4):
                wall[:, i * 1024 + c * 256:(i * 1024 + (c + 1) * 256)] = \
                    blk[:, c * 128:(c + 1) * 128].T
        woT = Wo[:, idx_v].T                   # [256 v, 512 embed]
        for c2 in range(2):
            wall[:, 3072 + c2 * 512:3072 + (c2 + 1) * 512] = \
                woT[c2 * 128:(c2 + 1) * 128, :]

        perm = np.zeros((128, 128), np.float16)
        for mm in range(128):
            perm[(mm // 64) * 64 + ((mm % 64) ^ 32), mm] = np.float16(1.0)
        m = {
            "xqT": xqTs[b], "xkT": xkTs[b], "xvT": xvTs[b],
            "wAll": wall.astype(ml_dtypes.bfloat16),
            "csq": csq, "csk": csk,
            "tri": tri.astype(ml_dtypes.bfloat16), "perm": perm,
        }
        if has_bias:
            wbias = np.empty((1, 768), np.float32)
            wbias[0, 0:256] = bq[idx_p]
            wbias[0, 256:512] = bk[idx_p]
            wbias[0, 512:768] = bv[idx_v]
            m["wB"] = wbias.astype(ml_dtypes.bfloat16)
        if use_mask:
            m["maskT"] = masks[b]
        in_maps.append(m)
    return in_maps


def classify_mask(attn_mask, key_padding_mask):
    am = np.asarray(attn_mask, np.float32)
    kp = np.asarray(key_padding_mask)
    if not kp.any():
        causal = np.where(
            np.tril(np.ones((L, L), bool)), np.float32(0.0),
            np.float32(NEG)).astype(np.float32)
        if np.array_equal(am, causal):
            return True, False
        if not am.any():
            return False, False
    return False, True


def kernel(query, key, value, key_padding_mask, attn_mask,
           Wq, bq, Wk, bk, Wv, bv, Wo, bo):
    causal, use_mask = classify_mask(attn_mask, key_padding_mask)
    has_bias = bool(np.asarray(bq).any() or np.asarray(bk).any()
                    or np.asarray(bv).any())
    nc = get_program(causal, use_mask, has_bias, reps=1)
    in_maps = _prep_in_maps(query, key, value, key_padding_mask, attn_mask,
                            Wq, bq, Wk, bk, Wv, bv, Wo, bo, use_mask, has_bias)
    res = run_bass_kernel_spmd(nc, in_maps, list(range(N_CORES)))
    bo = np.asarray(bo, np.float32)
    out = np.empty((B, L, EMBED), np.float32)
    for b in range(B):
        out[b] = (res.results[2 * b]["outp"]
                  + res.results[2 * b + 1]["outp"] + bo[None, :])
    return out
